# revision 19
# baseline (speedup 1.0000x reference)
"""Trainium2 Bass kernel for ChatGLM attention (S=2048, B=1, H=4096, NH=32, HD=128).

Sharding: tensor-parallel over heads across 8 NeuronCores (4 heads/core).
Each core computes its heads' QKV projection, RoPE, attention, and a
partial dense projection (contraction over its 512 hidden dims); the host
sums the 8 partials and adds the dense bias.

All matmuls run in bf16 (fp32 PSUM accumulation); RoPE tables and
elementwise arithmetic are fp32. Verified ~5e-3 scale-relative absmax
error vs the fp32 reference.
"""

import math
from contextlib import ExitStack

import ml_dtypes
import numpy as np

import concourse.bacc as bacc
import concourse.mybir as mybir
import concourse.tile as tile
from concourse.bass_utils import run_bass_kernel_spmd

S, B, H, NH, HD = 2048, 1, 4096, 32, 128
ROT = HD // 2
NCORES = 8
HPC = NH // NCORES          # heads per core = 4
QK_TILES = 2 * HPC          # q/k col tiles per core = 8
HT = H // 128               # hidden tiles = 32
ST = S // 128               # seq tiles = 16
SC = S // 512               # 512-wide seq chunks = 4

F32 = mybir.dt.float32
BF16 = mybir.dt.bfloat16
BF = ml_dtypes.bfloat16

_PROG_CACHE = {}


def _build_program(coeff: float, mode: str, debug: bool = False, phases: int = 3):
    """mode: 'causal' | 'none' | 'general'"""
    nc = bacc.Bacc("TRN2", target_bir_lowering=False, debug=False)

    # all weight-ish inputs are laid out partition-major by the host so each
    # logical tile load is one DMA with >=1KB contiguous bursts per partition
    hT = nc.dram_tensor("hT", [128, HT, S], BF16, kind="ExternalInput")
    w_qk = nc.dram_tensor("w_qk", [QK_TILES, 128, HT, 128], BF16, kind="ExternalInput")
    w_v = nc.dram_tensor("w_v", [128, HT, HPC * 128], BF16, kind="ExternalInput")
    b_qk = nc.dram_tensor("b_qk", [QK_TILES, 128], F32, kind="ExternalInput")
    b_v = nc.dram_tensor("b_v", [128, HPC * 128], F32, kind="ExternalInput")
    cs_q = nc.dram_tensor("cs_q", [128, 2, S], F32, kind="ExternalInput")   # cos, sin_eff
    cs_k = nc.dram_tensor("cs_k", [128, 2, S], F32, kind="ExternalInput")
    triu = nc.dram_tensor("triu", [128, 128], BF16, kind="ExternalInput")
    wdT = nc.dram_tensor("wdT", [HPC, 128, H], BF16, kind="ExternalInput")
    if mode == "general":
        m01 = nc.dram_tensor("m01", [ST, 128, S], BF16, kind="ExternalInput")
    F16 = mybir.dt.float16
    out_p = nc.dram_tensor("out_p", [S, H], F16, kind="ExternalOutput")
    if debug:
        dbg_q = nc.dram_tensor("dbg_q", [128, HPC, S], BF16, kind="ExternalOutput")
        dbg_k = nc.dram_tensor("dbg_k", [128, HPC, S], BF16, kind="ExternalOutput")
        dbg_v = nc.dram_tensor("dbg_v", [128, ST, HPC * 128], BF16, kind="ExternalOutput")
        dbg_p = nc.dram_tensor("dbg_p", [128, ST, S], BF16, kind="ExternalOutput")
        dbg_l = nc.dram_tensor("dbg_l", [1, S], F32, kind="ExternalOutput")
        dbg_c = nc.dram_tensor("dbg_c", [128, HPC, S], BF16, kind="ExternalOutput")

    def s0_of(t):
        return 128 * t if mode == "causal" else 0

    with tile.TileContext(nc) as tc, ExitStack() as ctx:
        const = ctx.enter_context(tc.tile_pool(name="const", bufs=1))
        psum = ctx.enter_context(tc.tile_pool(name="psum", bufs=4, space="PSUM"))
        scratch = ctx.enter_context(tc.tile_pool(name="scratch", bufs=2, space="DRAM"))

        b_qk_sb = const.tile([128, QK_TILES], F32)
        nc.sync.dma_start(b_qk_sb[:], b_qk[:].rearrange("t p -> p t"))
        b_v_sb = const.tile([128, HPC * 128], F32)
        nc.sync.dma_start(b_v_sb[:], b_v[:])
        triu_sb = const.tile([128, 128], BF16)
        nc.sync.dma_start(triu_sb[:], triu[:])
        ones_sb = const.tile([128, 1], BF16)
        nc.vector.memset(ones_sb[:], 1.0)
        qT_sb = const.tile([128, HPC, S], BF16)
        kT_sb = const.tile([128, HPC, S], BF16)
        v_sb = const.tile([128, ST, HPC * 128], BF16)

        # ---------------- Phase 1: QKV projection + RoPE ----------------
        with ExitStack() as qctx:
            wv_pool = qctx.enter_context(tc.tile_pool(name="wv", bufs=1))
            hid_pool = qctx.enter_context(tc.tile_pool(name="hid", bufs=1))
            cs_pool = qctx.enter_context(tc.tile_pool(name="cs", bufs=2))
            wqk_pool = qctx.enter_context(tc.tile_pool(name="wqk", bufs=3))
            rt_pool = qctx.enter_context(tc.tile_pool(name="rt", bufs=3))

            wv_sb = wv_pool.tile([128, HT, HPC * 128], BF16)
            nc.scalar.dma_start(wv_sb[:], w_v[:])

            for gc in range(SC):
                lo = gc * 512
                hid = hid_pool.tile([128, HT, 512], BF16, tag="hid")
                nc.sync.dma_start(hid[:, :HT // 2], hT[:, :HT // 2, lo:lo + 512])
                nc.scalar.dma_start(hid[:, HT // 2:], hT[:, HT // 2:, lo:lo + 512])

                # v part: [s, vcols] orientation
                for stl in range(4):
                    st = 4 * gc + stl
                    ps_v = psum.tile([128, 512], F32, tag="ps")
                    for ht in range(HT):
                        nc.tensor.matmul(
                            ps_v[:, : HPC * 128],
                            hid[:, ht, stl * 128:(stl + 1) * 128],
                            wv_sb[:, ht, :],
                            start=(ht == 0), stop=(ht == HT - 1),
                        )
                    nc.vector.tensor_tensor(
                        v_sb[:, st, :], ps_v[:, : HPC * 128], b_v_sb[:],
                        op=mybir.AluOpType.add,
                    )

                # cos/sin slices for this chunk (scaled tables for q)
                csq = cs_pool.tile([128, 2, 512], F32, tag="csq")
                nc.scalar.dma_start(csq[:], cs_q[:, :, lo:lo + 512])
                csk = cs_pool.tile([128, 2, 512], F32, tag="csk")
                nc.scalar.dma_start(csk[:], cs_k[:, :, lo:lo + 512])

                # q/k part: transposed orientation [d, s]
                for ct in range(QK_TILES):
                    n, is_k = ct // 2, ct % 2
                    wt = wqk_pool.tile([128, HT, 128], BF16, tag="wt")
                    (nc.sync if ct % 2 else nc.scalar).dma_start(wt[:], w_qk[ct])
                    cst = csk if is_k else csq
                    dest = kT_sb if is_k else qT_sb
                    ps_qk = psum.tile([128, 512], F32, tag="ps")
                    for ht in range(HT):
                        nc.tensor.matmul(
                            ps_qk[:],
                            wt[:, ht, :],
                            hid[:, ht, :],
                            start=(ht == 0), stop=(ht == HT - 1),
                        )
                    qf = rt_pool.tile([128, 512], F32, tag="qf")
                    nc.scalar.activation(
                        qf[:], ps_qk[:], mybir.ActivationFunctionType.Identity,
                        bias=b_qk_sb[:, ct:ct + 1],
                    )
                    qs = rt_pool.tile([128, 512], F32, tag="qs")
                    nc.gpsimd.dma_start(qs[0:32, :], qf[32:64, :])
                    nc.gpsimd.dma_start(qs[32:64, :], qf[0:32, :])
                    nc.gpsimd.dma_start(qs[64:96, :], qf[96:128, :])
                    nc.gpsimd.dma_start(qs[96:128, :], qf[64:96, :])
                    t1 = rt_pool.tile([128, 512], F32, tag="t1")
                    nc.gpsimd.tensor_tensor(
                        t1[:], qs[:], cst[:, 1, :], op=mybir.AluOpType.mult)
                    t2 = rt_pool.tile([128, 512], F32, tag="t2")
                    nc.vector.tensor_tensor(
                        t2[:], qf[:], cst[:, 0, :], op=mybir.AluOpType.mult)
                    nc.vector.tensor_tensor(
                        dest[:, n, lo:lo + 512], t1[:], t2[:],
                        op=mybir.AluOpType.add)

        if debug:
            nc.sync.dma_start(dbg_q[:], qT_sb[:])
            nc.sync.dma_start(dbg_k[:], kT_sb[:])
            nc.sync.dma_start(dbg_v[:], v_sb[:])

        # ---------------- Phase 2+3: attention + dense ----------------
        if phases < 2:
            nc.compile()
            return nc
        with ExitStack() as actx:
            attnw = actx.enter_context(tc.tile_pool(name="attnw", bufs=1))
            wdT_sb = attnw.tile([128, HPC, H], BF16)
            for n in range(HPC):
                nc.scalar.dma_start(wdT_sb[:, n, :], wdT[n])
            ctx_sb = attnw.tile([128, HPC, S], BF16)

            atmp_ctx = ExitStack()
            attn = atmp_ctx.enter_context(tc.tile_pool(name="attn", bufs=1))
            mt_pool = atmp_ctx.enter_context(tc.tile_pool(name="mt", bufs=2))
            ctx_psum = atmp_ctx.enter_context(tc.tile_pool(name="ctxps", bufs=1, space="PSUM"))

            for n in range(HPC):
                ctx_done = []
                for h2 in range(2):
                    sLo, sHi = 1024 * h2, 1024 * (h2 + 1)
                    PTh = attn.tile([128, ST, 1024], BF16, tag="PT", bufs=2, name="PTh")
                    for t in range(ST):
                        t0 = s0_of(t)
                        if t0 >= sHi:
                            continue
                        s = max(t0, sLo)
                        while s < sHi:
                            w = min(512, sHi - s)
                            ps_s = psum.tile([128, 512], F32, tag="ps", name="ps_s")
                            nc.tensor.matmul(
                                ps_s[:, :w],
                                kT_sb[:, n, 128 * t:128 * (t + 1)],
                                qT_sb[:, n, s:s + w],
                                start=True, stop=True,
                            )
                            nc.scalar.activation(
                                PTh[:, t, s - sLo:s - sLo + w], ps_s[:, :w],
                                mybir.ActivationFunctionType.Exp, scale=coeff)
                            s += w
                        if mode == "causal" and sLo <= t0:
                            if t0 % 512:
                                c0 = t0 - (t0 % 512)
                                nc.vector.memset(PTh[:, t, c0 - sLo:t0 - sLo], 0.0)
                            nc.vector.tensor_tensor(
                                PTh[:, t, t0 - sLo:t0 - sLo + 128],
                                PTh[:, t, t0 - sLo:t0 - sLo + 128], triu_sb[:],
                                op=mybir.AluOpType.mult)
                        elif mode == "general":
                            mt = mt_pool.tile([128, 1024], BF16, tag="m01")
                            nc.sync.dma_start(mt[:], m01[t][:, sLo:sHi])
                            nc.vector.tensor_tensor(
                                PTh[:, t, :], PTh[:, t, :], mt[:],
                                op=mybir.AluOpType.mult)

                    # row sums for this half, then 1/l broadcast
                    l_sb = attn.tile([1, 1024], F32, tag="l", bufs=2, name="l_sb")
                    for cc in range(2):
                        c = 2 * h2 + cc
                        tmax = min(ST - 1, 4 * c + 3) if mode == "causal" else ST - 1
                        ps_l = psum.tile([1, 512], F32, tag="ps", name="ps_l")
                        for t in range(tmax + 1):
                            nc.tensor.matmul(
                                ps_l[:], ones_sb[:], PTh[:, t, 512 * cc:512 * (cc + 1)],
                                start=(t == 0), stop=(t == tmax))
                        nc.scalar.copy(l_sb[:, 512 * cc:512 * (cc + 1)], ps_l[:])
                    linv = attn.tile([1, 1024], F32, tag="linv", bufs=2, name="linv")
                    nc.vector.reciprocal(linv[:], l_sb[:])
                    l_dram = scratch.tile([1, 1024], F32, tag="ldram")
                    nc.sync.dma_start(l_dram[:], linv[:])
                    linv_b = attn.tile([128, 1024], F32, tag="linvb", bufs=2, name="linv_b")
                    nc.scalar.dma_start(linv_b[:], l_dram[:].to_broadcast((128, 1024)))

                    # ctxT[d, s] accumulation over t tiles (this half)
                    ctx_ps = ctx_psum.tile([128, 1024], F32, tag="ctx", bufs=2, name="ctx_ps")
                    for cc in range(2):
                        c = 2 * h2 + cc
                        tmax = min(ST - 1, 4 * c + 3) if mode == "causal" else ST - 1
                        for t in range(tmax + 1):
                            nc.tensor.matmul(
                                ctx_ps[:, 512 * cc:512 * (cc + 1)],
                                v_sb[:, t, 128 * n:128 * (n + 1)],
                                PTh[:, t, 512 * cc:512 * (cc + 1)],
                                start=(t == 0), stop=(t == tmax))
                    nc.vector.tensor_tensor(
                        ctx_sb[:, n, sLo:sHi], ctx_ps[:], linv_b[:],
                        op=mybir.AluOpType.mult)
                if debug and n == 0:
                    pass

            if debug:
                nc.sync.dma_start(dbg_c[:], ctx_sb[:])

            atmp_ctx.close()
            if phases < 3:
                nc.compile()
                return nc

            # dense partial: out[s, ho] += ctxT[:, n, s].T @ wdT[n][:, ho]
            with ExitStack() as dctx:
                dout = dctx.enter_context(tc.tile_pool(name="dout", bufs=3))
                for st in range(ST):
                    ot = dout.tile([128, 8, 512], F16, tag="ot")
                    for cg in range(2):
                        ps_o = [psum.tile([128, 512], F32, tag="ps", name=f"ps_o{i}")
                                for i in range(4)]
                        for n in range(HPC):
                            for cc in range(4):
                                ch = cg * 4 + cc
                                nc.tensor.matmul(
                                    ps_o[cc][:],
                                    ctx_sb[:, n, 128 * st:128 * (st + 1)],
                                    wdT_sb[:, n, 512 * ch:512 * (ch + 1)],
                                    start=(n == 0), stop=(n == HPC - 1))
                        for cc in range(4):
                            ch = cg * 4 + cc
                            if cc % 2 == 0:
                                nc.scalar.copy(ot[:, ch, :], ps_o[cc][:])
                            else:
                                nc.vector.tensor_copy(ot[:, ch, :], ps_o[cc][:])
                    eng = nc.scalar if st % 2 == 0 else nc.sync
                    eng.dma_start(
                        out_p[128 * st:128 * (st + 1), :],
                        ot[:].rearrange("p c s -> p (c s)"))

    nc.compile()
    return nc


def _prep_inputs(hidden_states, position_ids, attention_mask, layer_id, Wqkv, bqkv, Wd):
    coeff = float(np.asarray(layer_id).item() + 1)
    m = np.asarray(attention_mask).reshape(S, S)
    if not m.any():
        mode = "none"
    elif bool((m == np.triu(np.ones((S, S), bool), 1)).all()):
        mode = "causal"
    else:
        mode = "general"

    hs = np.asarray(hidden_states, np.float32).reshape(S, H)
    hT = np.ascontiguousarray(hs.T.reshape(HT, 128, S).transpose(1, 0, 2)).astype(BF)

    # RoPE tables (match reference fp32 math)
    inv_freq = (1.0 / (10000.0 ** (np.arange(0, ROT, 2, dtype=np.float32) / ROT))).astype(np.float32)
    freqs = np.arange(S, dtype=np.float32)[:, None] * inv_freq[None, :]
    emb = np.concatenate([freqs, freqs], axis=-1)          # [S, 64]
    cos_t, sin_t = np.cos(emb), np.sin(emb)                # fp32 [S, 64]
    pid = np.asarray(position_ids)[0, 0, :].astype(np.int64)
    bid = np.asarray(position_ids)[0, 1, :].astype(np.int64)
    cp, sp = cos_t[pid].T, sin_t[pid].T                    # [64, S]
    cb, sb_ = cos_t[bid].T, sin_t[bid].T
    cos_full = np.concatenate([cp, cb], 0)                 # [128, S]
    # rope via partition-swapped copy: t1[p] = q[p^32] * sin_eff[p]
    sin_eff = np.concatenate([-sp[:32], sp[32:], -sb_[:32], sb_[32:]], 0)
    qscale = np.float32(1.0 / (math.sqrt(HD) * coeff))
    cs_q = np.stack([cos_full * qscale, sin_eff * qscale], 1).astype(np.float32)  # [128,2,S]
    cs_k = np.stack([cos_full, sin_eff], 1).astype(np.float32)

    tri = np.triu(np.ones((128, 128), np.float32)).astype(BF)  # t<=s allowed

    Wq = np.asarray(Wqkv, np.float32).reshape(NH, 3, HD, H)
    bq = np.asarray(bqkv, np.float32).reshape(NH, 3, HD)
    Wd_ = np.asarray(Wd, np.float32)

    per_core = []
    for c in range(NCORES):
        heads = slice(4 * c, 4 * c + 4)
        wqk = Wq[heads, 0:2]                               # [4, 2, 128, H]
        # -> [ct=8, p=128, ht=32, j=128]
        wqk = wqk.reshape(8, HD, HT, 128).transpose(0, 3, 2, 1)   # [8, p, ht, j]
        w_qk = np.ascontiguousarray(wqk).astype(BF)
        wv = Wq[heads, 2]                                  # [4, 128, H]
        wv = wv.reshape(4 * HD, HT, 128).transpose(1, 2, 0)  # [ht, p, 512]
        wv = wv.transpose(1, 0, 2)                         # [p, ht, 512]
        w_v = np.ascontiguousarray(wv).astype(BF)
        b_qk = np.ascontiguousarray(bq[heads, 0:2].reshape(8, 128)).astype(np.float32)
        b_v = np.broadcast_to(bq[heads, 2].reshape(1, 512), (128, 512)).astype(np.float32)
        wd = Wd_[:, 512 * c:512 * (c + 1)]                 # [H, 512]
        wd = np.ascontiguousarray(wd.T.reshape(HPC, 128, H)).astype(BF)
        im = {
            "hT": hT, "w_qk": w_qk, "w_v": w_v, "b_qk": b_qk, "b_v": b_v,
            "cs_q": cs_q, "cs_k": cs_k, "triu": tri, "wdT": wd,
        }
        if mode == "general":
            m01 = (~m).astype(np.float32).T  # [t, s] 1=allowed
            im["m01"] = np.ascontiguousarray(m01.reshape(ST, 128, S)).astype(BF)
        per_core.append(im)
    return coeff, mode, per_core


def kernel(hidden_states, position_ids, attention_mask, layer_id, Wqkv, bqkv, Wd, bd):
    coeff, mode, per_core = _prep_inputs(
        hidden_states, position_ids, attention_mask, layer_id, Wqkv, bqkv, Wd)
    key = (coeff, mode)
    if key not in _PROG_CACHE:
        _PROG_CACHE[key] = _build_program(coeff, mode)
    nc = _PROG_CACHE[key]
    res = run_bass_kernel_spmd(nc, per_core, core_ids=list(range(NCORES)))
    out = np.zeros((S, H), np.float64)
    for r in res.results:
        out += r["out_p"].astype(np.float64)
    out += np.asarray(bd, np.float32).astype(np.float64)
    return out.astype(np.float32).reshape(S, B, H)


# revision 23
# speedup vs baseline: 1.0792x; 1.0792x over previous
"""Trainium2 Bass kernel for ChatGLM attention (S=2048, B=1, H=4096, NH=32, HD=128).

Sharding: tensor-parallel over heads across 8 NeuronCores (4 heads/core).
Each core computes its heads' QKV projection, RoPE, attention, and a
partial dense projection (contraction over its 512 hidden dims); the host
sums the 8 partials and adds the dense bias.

All matmuls run in bf16 (fp32 PSUM accumulation); RoPE tables and
elementwise arithmetic are fp32. Verified ~5e-3 scale-relative absmax
error vs the fp32 reference.
"""

import math
from contextlib import ExitStack

import ml_dtypes
import numpy as np

import concourse.bacc as bacc
import concourse.mybir as mybir
import concourse.tile as tile
from concourse.bass_utils import run_bass_kernel_spmd

S, B, H, NH, HD = 2048, 1, 4096, 32, 128
ROT = HD // 2
NCORES = 8
HPC = NH // NCORES          # heads per core = 4
QK_TILES = 2 * HPC          # q/k col tiles per core = 8
HT = H // 128               # hidden tiles = 32
ST = S // 128               # seq tiles = 16
SC = S // 512               # 512-wide seq chunks = 4

F32 = mybir.dt.float32
BF16 = mybir.dt.bfloat16
BF = ml_dtypes.bfloat16

_PROG_CACHE = {}


def _build_program(coeff: float, mode: str, debug: bool = False, phases: int = 3):
    """mode: 'causal' | 'none' | 'general'"""
    nc = bacc.Bacc("TRN2", target_bir_lowering=False, debug=False)

    # all weight-ish inputs are laid out partition-major by the host so each
    # logical tile load is one DMA with >=1KB contiguous bursts per partition
    hT = nc.dram_tensor("hT", [128, HT, S], BF16, kind="ExternalInput")
    w_qk = nc.dram_tensor("w_qk", [QK_TILES, 128, HT, 128], BF16, kind="ExternalInput")
    w_v = nc.dram_tensor("w_v", [128, HT, HPC * 128], BF16, kind="ExternalInput")
    b_qk = nc.dram_tensor("b_qk", [QK_TILES, 128], F32, kind="ExternalInput")
    b_v = nc.dram_tensor("b_v", [128, HPC * 128], F32, kind="ExternalInput")
    cs_q = nc.dram_tensor("cs_q", [128, 2, S], F32, kind="ExternalInput")   # cos, sin_eff
    cs_k = nc.dram_tensor("cs_k", [128, 2, S], F32, kind="ExternalInput")
    triu = nc.dram_tensor("triu", [128, 128], BF16, kind="ExternalInput")
    wdT = nc.dram_tensor("wdT", [HPC, 128, H], BF16, kind="ExternalInput")
    if mode == "general":
        m01 = nc.dram_tensor("m01", [ST, 128, S], BF16, kind="ExternalInput")
    F16 = mybir.dt.float16
    out_p = nc.dram_tensor("out_p", [S, H], F16, kind="ExternalOutput")
    if debug:
        dbg_q = nc.dram_tensor("dbg_q", [128, HPC, S], BF16, kind="ExternalOutput")
        dbg_k = nc.dram_tensor("dbg_k", [128, HPC, S], BF16, kind="ExternalOutput")
        dbg_v = nc.dram_tensor("dbg_v", [128, ST, HPC * 128], BF16, kind="ExternalOutput")
        dbg_p = nc.dram_tensor("dbg_p", [128, ST, S], BF16, kind="ExternalOutput")
        dbg_l = nc.dram_tensor("dbg_l", [1, S], F32, kind="ExternalOutput")
        dbg_c = nc.dram_tensor("dbg_c", [128, HPC, S], BF16, kind="ExternalOutput")

    def s0_of(t):
        return 128 * t if mode == "causal" else 0

    with tile.TileContext(nc) as tc, ExitStack() as ctx:
        const = ctx.enter_context(tc.tile_pool(name="const", bufs=1))
        psum = ctx.enter_context(tc.tile_pool(name="psum", bufs=4, space="PSUM"))
        scratch = ctx.enter_context(tc.tile_pool(name="scratch", bufs=2, space="DRAM"))

        b_qk_sb = const.tile([128, QK_TILES], F32)
        nc.sync.dma_start(b_qk_sb[:], b_qk[:].rearrange("t p -> p t"))
        b_v_sb = const.tile([128, HPC * 128], F32)
        nc.sync.dma_start(b_v_sb[:], b_v[:])
        triu_sb = const.tile([128, 128], BF16)
        nc.sync.dma_start(triu_sb[:], triu[:])
        ones_sb = const.tile([128, 1], BF16)
        nc.vector.memset(ones_sb[:], 1.0)
        qT_sb = const.tile([128, HPC, S], BF16)
        kT_sb = const.tile([128, HPC, S], BF16)
        v_sb = const.tile([128, ST, HPC * 128], BF16)

        # ---------------- Phase 1: QKV projection + RoPE ----------------
        with ExitStack() as qctx:
            wv_pool = qctx.enter_context(tc.tile_pool(name="wv", bufs=1))
            hid_pool = qctx.enter_context(tc.tile_pool(name="hid", bufs=2))
            cs_pool = qctx.enter_context(tc.tile_pool(name="cs", bufs=1))
            wqk_pool = qctx.enter_context(tc.tile_pool(name="wqk", bufs=2))
            rt_pool = qctx.enter_context(tc.tile_pool(name="rt", bufs=2))

            wv_sb = wv_pool.tile([128, HT, HPC * 128], BF16)
            nc.scalar.dma_start(wv_sb[:], w_v[:])

            for gc in range(SC):
                lo = gc * 512
                hidA = hid_pool.tile([128, HT // 2, 512], BF16, tag="hidA")
                nc.sync.dma_start(hidA[:], hT[:, :HT // 2, lo:lo + 512])
                hidB = hid_pool.tile([128, HT // 2, 512], BF16, tag="hidB")
                nc.scalar.dma_start(hidB[:], hT[:, HT // 2:, lo:lo + 512])

                def hid_slice(ht, js, je):
                    t = hidA if ht < HT // 2 else hidB
                    return t[:, ht % (HT // 2), js:je]

                # v part: [s, vcols] orientation
                for stl in range(4):
                    st = 4 * gc + stl
                    ps_v = psum.tile([128, 512], F32, tag="ps")
                    for ht in range(HT):
                        nc.tensor.matmul(
                            ps_v[:, : HPC * 128],
                            hid_slice(ht, stl * 128, (stl + 1) * 128),
                            wv_sb[:, ht, :],
                            start=(ht == 0), stop=(ht == HT - 1),
                        )
                    nc.vector.tensor_tensor(
                        v_sb[:, st, :], ps_v[:, : HPC * 128], b_v_sb[:],
                        op=mybir.AluOpType.add,
                    )

                # cos/sin slices for this chunk (scaled tables for q)
                csq = cs_pool.tile([128, 2, 512], F32, tag="csq")
                nc.scalar.dma_start(csq[:], cs_q[:, :, lo:lo + 512])
                csk = cs_pool.tile([128, 2, 512], F32, tag="csk")
                nc.scalar.dma_start(csk[:], cs_k[:, :, lo:lo + 512])

                # q/k part: transposed orientation [d, s]
                for ct in range(QK_TILES):
                    n, is_k = ct // 2, ct % 2
                    wt = wqk_pool.tile([128, HT, 128], BF16, tag="wt")
                    (nc.sync if ct % 2 else nc.scalar).dma_start(wt[:], w_qk[ct])
                    cst = csk if is_k else csq
                    dest = kT_sb if is_k else qT_sb
                    ps_qk = psum.tile([128, 512], F32, tag="ps")
                    for ht in range(HT):
                        nc.tensor.matmul(
                            ps_qk[:],
                            wt[:, ht, :],
                            hid_slice(ht, 0, 512),
                            start=(ht == 0), stop=(ht == HT - 1),
                        )
                    qf = rt_pool.tile([128, 512], F32, tag="qf")
                    nc.scalar.activation(
                        qf[:], ps_qk[:], mybir.ActivationFunctionType.Identity,
                        bias=b_qk_sb[:, ct:ct + 1],
                    )
                    qs = rt_pool.tile([128, 512], F32, tag="qs")
                    nc.gpsimd.dma_start(qs[0:32, :], qf[32:64, :])
                    nc.gpsimd.dma_start(qs[32:64, :], qf[0:32, :])
                    nc.gpsimd.dma_start(qs[64:96, :], qf[96:128, :])
                    nc.gpsimd.dma_start(qs[96:128, :], qf[64:96, :])
                    t1 = rt_pool.tile([128, 512], F32, tag="t1")
                    nc.gpsimd.tensor_tensor(
                        t1[:], qs[:], cst[:, 1, :], op=mybir.AluOpType.mult)
                    t2 = rt_pool.tile([128, 512], F32, tag="t2")
                    nc.vector.tensor_tensor(
                        t2[:], qf[:], cst[:, 0, :], op=mybir.AluOpType.mult)
                    nc.vector.tensor_tensor(
                        dest[:, n, lo:lo + 512], t1[:], t2[:],
                        op=mybir.AluOpType.add)

        if debug:
            nc.sync.dma_start(dbg_q[:], qT_sb[:])
            nc.sync.dma_start(dbg_k[:], kT_sb[:])
            nc.sync.dma_start(dbg_v[:], v_sb[:])

        # ---------------- Phase 2+3: attention + dense ----------------
        if phases < 2:
            nc.compile()
            return nc
        with ExitStack() as actx:
            attnw = actx.enter_context(tc.tile_pool(name="attnw", bufs=1))
            wdT_sb = attnw.tile([128, HPC, H], BF16)
            for n in range(HPC):
                nc.scalar.dma_start(wdT_sb[:, n, :], wdT[n])
            ctx_sb = attnw.tile([128, HPC, S], BF16)

            atmp_ctx = ExitStack()
            attn = atmp_ctx.enter_context(tc.tile_pool(name="attn", bufs=1))
            mt_pool = atmp_ctx.enter_context(tc.tile_pool(name="mt", bufs=2))
            ctx_psum = atmp_ctx.enter_context(tc.tile_pool(name="ctxps", bufs=1, space="PSUM"))

            for n in range(HPC):
                ctx_done = []
                for h2 in range(2):
                    sLo, sHi = 1024 * h2, 1024 * (h2 + 1)
                    PTh = attn.tile([128, ST, 1024], BF16, tag="PT", bufs=2, name="PTh")
                    for t in range(ST):
                        t0 = s0_of(t)
                        if t0 >= sHi:
                            continue
                        s = max(t0, sLo)
                        while s < sHi:
                            w = min(512, sHi - s)
                            ps_s = psum.tile([128, 512], F32, tag="ps", name="ps_s")
                            nc.tensor.matmul(
                                ps_s[:, :w],
                                kT_sb[:, n, 128 * t:128 * (t + 1)],
                                qT_sb[:, n, s:s + w],
                                start=True, stop=True,
                            )
                            nc.scalar.activation(
                                PTh[:, t, s - sLo:s - sLo + w], ps_s[:, :w],
                                mybir.ActivationFunctionType.Exp, scale=coeff)
                            s += w
                        if mode == "causal" and sLo <= t0:
                            if t0 % 512:
                                c0 = t0 - (t0 % 512)
                                nc.vector.memset(PTh[:, t, c0 - sLo:t0 - sLo], 0.0)
                            nc.vector.tensor_tensor(
                                PTh[:, t, t0 - sLo:t0 - sLo + 128],
                                PTh[:, t, t0 - sLo:t0 - sLo + 128], triu_sb[:],
                                op=mybir.AluOpType.mult)
                        elif mode == "general":
                            mt = mt_pool.tile([128, 1024], BF16, tag="m01")
                            nc.sync.dma_start(mt[:], m01[t][:, sLo:sHi])
                            nc.vector.tensor_tensor(
                                PTh[:, t, :], PTh[:, t, :], mt[:],
                                op=mybir.AluOpType.mult)

                    # row sums for this half, then 1/l broadcast
                    l_sb = attn.tile([1, 1024], F32, tag="l", bufs=2, name="l_sb")
                    for cc in range(2):
                        c = 2 * h2 + cc
                        tmax = min(ST - 1, 4 * c + 3) if mode == "causal" else ST - 1
                        ps_l = psum.tile([1, 512], F32, tag="ps", name="ps_l")
                        for t in range(tmax + 1):
                            nc.tensor.matmul(
                                ps_l[:], ones_sb[:], PTh[:, t, 512 * cc:512 * (cc + 1)],
                                start=(t == 0), stop=(t == tmax))
                        nc.scalar.copy(l_sb[:, 512 * cc:512 * (cc + 1)], ps_l[:])
                    linv = attn.tile([1, 1024], F32, tag="linv", bufs=2, name="linv")
                    nc.vector.reciprocal(linv[:], l_sb[:])
                    l_dram = scratch.tile([1, 1024], F32, tag="ldram")
                    nc.sync.dma_start(l_dram[:], linv[:])
                    linv_b = attn.tile([128, 1024], F32, tag="linvb", bufs=2, name="linv_b")
                    nc.scalar.dma_start(linv_b[:], l_dram[:].to_broadcast((128, 1024)))

                    # ctxT[d, s] accumulation over t tiles (this half)
                    ctx_ps = ctx_psum.tile([128, 1024], F32, tag="ctx", bufs=2, name="ctx_ps")
                    for cc in range(2):
                        c = 2 * h2 + cc
                        tmax = min(ST - 1, 4 * c + 3) if mode == "causal" else ST - 1
                        for t in range(tmax + 1):
                            nc.tensor.matmul(
                                ctx_ps[:, 512 * cc:512 * (cc + 1)],
                                v_sb[:, t, 128 * n:128 * (n + 1)],
                                PTh[:, t, 512 * cc:512 * (cc + 1)],
                                start=(t == 0), stop=(t == tmax))
                    nc.vector.tensor_tensor(
                        ctx_sb[:, n, sLo:sHi], ctx_ps[:], linv_b[:],
                        op=mybir.AluOpType.mult)
                if debug and n == 0:
                    pass

            if debug:
                nc.sync.dma_start(dbg_c[:], ctx_sb[:])

            atmp_ctx.close()
            if phases < 3:
                nc.compile()
                return nc

            # dense partial: out[s, ho] += ctxT[:, n, s].T @ wdT[n][:, ho]
            with ExitStack() as dctx:
                dout = dctx.enter_context(tc.tile_pool(name="dout", bufs=3))
                for st in range(ST):
                    ot = dout.tile([128, 8, 512], F16, tag="ot")
                    for cg in range(2):
                        ps_o = [psum.tile([128, 512], F32, tag="ps", name=f"ps_o{i}")
                                for i in range(4)]
                        for n in range(HPC):
                            for cc in range(4):
                                ch = cg * 4 + cc
                                nc.tensor.matmul(
                                    ps_o[cc][:],
                                    ctx_sb[:, n, 128 * st:128 * (st + 1)],
                                    wdT_sb[:, n, 512 * ch:512 * (ch + 1)],
                                    start=(n == 0), stop=(n == HPC - 1))
                        for cc in range(4):
                            ch = cg * 4 + cc
                            if cc % 2 == 0:
                                nc.scalar.copy(ot[:, ch, :], ps_o[cc][:])
                            else:
                                nc.vector.tensor_copy(ot[:, ch, :], ps_o[cc][:])
                    eng = nc.scalar if st % 2 == 0 else nc.sync
                    eng.dma_start(
                        out_p[128 * st:128 * (st + 1), :],
                        ot[:].rearrange("p c s -> p (c s)"))

    nc.compile()
    return nc


def _prep_inputs(hidden_states, position_ids, attention_mask, layer_id, Wqkv, bqkv, Wd):
    coeff = float(np.asarray(layer_id).item() + 1)
    m = np.asarray(attention_mask).reshape(S, S)
    if not m.any():
        mode = "none"
    elif bool((m == np.triu(np.ones((S, S), bool), 1)).all()):
        mode = "causal"
    else:
        mode = "general"

    hs = np.asarray(hidden_states, np.float32).reshape(S, H)
    hT = np.ascontiguousarray(hs.T.reshape(HT, 128, S).transpose(1, 0, 2)).astype(BF)

    # RoPE tables (match reference fp32 math)
    inv_freq = (1.0 / (10000.0 ** (np.arange(0, ROT, 2, dtype=np.float32) / ROT))).astype(np.float32)
    freqs = np.arange(S, dtype=np.float32)[:, None] * inv_freq[None, :]
    emb = np.concatenate([freqs, freqs], axis=-1)          # [S, 64]
    cos_t, sin_t = np.cos(emb), np.sin(emb)                # fp32 [S, 64]
    pid = np.asarray(position_ids)[0, 0, :].astype(np.int64)
    bid = np.asarray(position_ids)[0, 1, :].astype(np.int64)
    cp, sp = cos_t[pid].T, sin_t[pid].T                    # [64, S]
    cb, sb_ = cos_t[bid].T, sin_t[bid].T
    cos_full = np.concatenate([cp, cb], 0)                 # [128, S]
    # rope via partition-swapped copy: t1[p] = q[p^32] * sin_eff[p]
    sin_eff = np.concatenate([-sp[:32], sp[32:], -sb_[:32], sb_[32:]], 0)
    qscale = np.float32(1.0 / (math.sqrt(HD) * coeff))
    cs_q = np.stack([cos_full * qscale, sin_eff * qscale], 1).astype(np.float32)  # [128,2,S]
    cs_k = np.stack([cos_full, sin_eff], 1).astype(np.float32)

    tri = np.triu(np.ones((128, 128), np.float32)).astype(BF)  # t<=s allowed

    Wq = np.asarray(Wqkv, np.float32).reshape(NH, 3, HD, H)
    bq = np.asarray(bqkv, np.float32).reshape(NH, 3, HD)
    Wd_ = np.asarray(Wd, np.float32)

    per_core = []
    for c in range(NCORES):
        heads = slice(4 * c, 4 * c + 4)
        wqk = Wq[heads, 0:2]                               # [4, 2, 128, H]
        # -> [ct=8, p=128, ht=32, j=128]
        wqk = wqk.reshape(8, HD, HT, 128).transpose(0, 3, 2, 1)   # [8, p, ht, j]
        w_qk = np.ascontiguousarray(wqk).astype(BF)
        wv = Wq[heads, 2]                                  # [4, 128, H]
        wv = wv.reshape(4 * HD, HT, 128).transpose(1, 2, 0)  # [ht, p, 512]
        wv = wv.transpose(1, 0, 2)                         # [p, ht, 512]
        w_v = np.ascontiguousarray(wv).astype(BF)
        b_qk = np.ascontiguousarray(bq[heads, 0:2].reshape(8, 128)).astype(np.float32)
        b_v = np.broadcast_to(bq[heads, 2].reshape(1, 512), (128, 512)).astype(np.float32)
        wd = Wd_[:, 512 * c:512 * (c + 1)]                 # [H, 512]
        wd = np.ascontiguousarray(wd.T.reshape(HPC, 128, H)).astype(BF)
        im = {
            "hT": hT, "w_qk": w_qk, "w_v": w_v, "b_qk": b_qk, "b_v": b_v,
            "cs_q": cs_q, "cs_k": cs_k, "triu": tri, "wdT": wd,
        }
        if mode == "general":
            m01 = (~m).astype(np.float32).T  # [t, s] 1=allowed
            im["m01"] = np.ascontiguousarray(m01.reshape(ST, 128, S)).astype(BF)
        per_core.append(im)
    return coeff, mode, per_core


def kernel(hidden_states, position_ids, attention_mask, layer_id, Wqkv, bqkv, Wd, bd):
    coeff, mode, per_core = _prep_inputs(
        hidden_states, position_ids, attention_mask, layer_id, Wqkv, bqkv, Wd)
    key = (coeff, mode)
    if key not in _PROG_CACHE:
        _PROG_CACHE[key] = _build_program(coeff, mode)
    nc = _PROG_CACHE[key]
    res = run_bass_kernel_spmd(nc, per_core, core_ids=list(range(NCORES)))
    out = np.zeros((S, H), np.float64)
    for r in res.results:
        out += r["out_p"].astype(np.float64)
    out += np.asarray(bd, np.float32).astype(np.float64)
    return out.astype(np.float32).reshape(S, B, H)


# revision 27
# speedup vs baseline: 1.1092x; 1.0278x over previous
"""Trainium2 Bass kernel for ChatGLM attention (S=2048, B=1, H=4096, NH=32, HD=128).

Sharding: tensor-parallel over heads across 8 NeuronCores (4 heads/core).
Each core computes its heads' QKV projection, RoPE, attention, and a
partial dense projection (contraction over its 512 hidden dims); the host
sums the 8 partials and adds the dense bias.

All matmuls run in bf16 (fp32 PSUM accumulation); RoPE tables and
elementwise arithmetic are fp32. Verified ~5e-3 scale-relative absmax
error vs the fp32 reference.
"""

import math
from contextlib import ExitStack

import ml_dtypes
import numpy as np

import concourse.bacc as bacc
import concourse.mybir as mybir
import concourse.tile as tile
from concourse.bass_utils import run_bass_kernel_spmd

S, B, H, NH, HD = 2048, 1, 4096, 32, 128
ROT = HD // 2
NCORES = 8
HPC = NH // NCORES          # heads per core = 4
QK_TILES = 2 * HPC          # q/k col tiles per core = 8
HT = H // 128               # hidden tiles = 32
ST = S // 128               # seq tiles = 16
SC = S // 512               # 512-wide seq chunks = 4

F32 = mybir.dt.float32
BF16 = mybir.dt.bfloat16
BF = ml_dtypes.bfloat16

_PROG_CACHE = {}


def _build_program(coeff: float, mode: str, debug: bool = False, phases: int = 3):
    """mode: 'causal' | 'none' | 'general'"""
    nc = bacc.Bacc("TRN2", target_bir_lowering=False, debug=False)

    # all weight-ish inputs are laid out partition-major by the host so each
    # logical tile load is one DMA with >=1KB contiguous bursts per partition
    hT = nc.dram_tensor("hT", [128, HT, S], BF16, kind="ExternalInput")
    w_qk = nc.dram_tensor("w_qk", [QK_TILES, 128, HT, 128], BF16, kind="ExternalInput")
    w_v = nc.dram_tensor("w_v", [128, HT, HPC * 128], BF16, kind="ExternalInput")
    b_qk = nc.dram_tensor("b_qk", [QK_TILES, 128], F32, kind="ExternalInput")
    b_v = nc.dram_tensor("b_v", [128, HPC * 128], F32, kind="ExternalInput")
    cs_q = nc.dram_tensor("cs_q", [128, 2, S], F32, kind="ExternalInput")   # cos, sin_eff
    cs_k = nc.dram_tensor("cs_k", [128, 2, S], F32, kind="ExternalInput")
    triu = nc.dram_tensor("triu", [128, 128], BF16, kind="ExternalInput")
    wdT = nc.dram_tensor("wdT", [HPC, 128, H], BF16, kind="ExternalInput")
    if mode == "general":
        m01 = nc.dram_tensor("m01", [ST, 128, S], BF16, kind="ExternalInput")
    F16 = mybir.dt.float16
    out_p = nc.dram_tensor("out_p", [S, H], F16, kind="ExternalOutput")
    if debug:
        dbg_q = nc.dram_tensor("dbg_q", [128, HPC, S], BF16, kind="ExternalOutput")
        dbg_k = nc.dram_tensor("dbg_k", [128, HPC, S], BF16, kind="ExternalOutput")
        dbg_v = nc.dram_tensor("dbg_v", [128, ST, HPC * 128], BF16, kind="ExternalOutput")
        dbg_p = nc.dram_tensor("dbg_p", [128, ST, S], BF16, kind="ExternalOutput")
        dbg_l = nc.dram_tensor("dbg_l", [1, S], F32, kind="ExternalOutput")
        dbg_c = nc.dram_tensor("dbg_c", [128, HPC, S], BF16, kind="ExternalOutput")

    def s0_of(t):
        return 128 * t if mode == "causal" else 0

    with tile.TileContext(nc) as tc, ExitStack() as ctx:
        const = ctx.enter_context(tc.tile_pool(name="const", bufs=1))
        psum = ctx.enter_context(tc.tile_pool(name="psum", bufs=4, space="PSUM"))
        scratch = ctx.enter_context(tc.tile_pool(name="scratch", bufs=2, space="DRAM"))

        b_qk_sb = const.tile([128, QK_TILES], F32)
        nc.sync.dma_start(b_qk_sb[:], b_qk[:].rearrange("t p -> p t"))
        b_v_sb = const.tile([128, HPC * 128], F32)
        nc.sync.dma_start(b_v_sb[:], b_v[:])
        triu_sb = const.tile([128, 128], BF16)
        nc.sync.dma_start(triu_sb[:], triu[:])
        ones_sb = const.tile([128, 1], BF16)
        nc.vector.memset(ones_sb[:], 1.0)
        qT_sb = const.tile([128, HPC, S], BF16)
        kT_sb = const.tile([128, HPC, S], BF16)
        v_sb = const.tile([128, ST, HPC * 128], BF16)

        # ---------------- Phase 1: QKV projection + RoPE ----------------
        with ExitStack() as qctx:
            wv_pool = qctx.enter_context(tc.tile_pool(name="wv", bufs=1))
            hid_pool = qctx.enter_context(tc.tile_pool(name="hid", bufs=2))
            cs_pool = qctx.enter_context(tc.tile_pool(name="cs", bufs=1))
            wqk_pool = qctx.enter_context(tc.tile_pool(name="wqk", bufs=2))
            rt_pool = qctx.enter_context(tc.tile_pool(name="rt", bufs=2))

            wv_sb = wv_pool.tile([128, HT, HPC * 128], BF16)
            nc.scalar.dma_start(wv_sb[:], w_v[:])

            for gc in range(SC):
                lo = gc * 512
                hidA = hid_pool.tile([128, HT // 2, 512], BF16, tag="hidA")
                nc.sync.dma_start(hidA[:], hT[:, :HT // 2, lo:lo + 512])
                hidB = hid_pool.tile([128, HT // 2, 512], BF16, tag="hidB")
                nc.scalar.dma_start(hidB[:], hT[:, HT // 2:, lo:lo + 512])

                def hid_slice(ht, js, je):
                    t = hidA if ht < HT // 2 else hidB
                    return t[:, ht % (HT // 2), js:je]

                # v part: [s, vcols] orientation
                for stl in range(4):
                    st = 4 * gc + stl
                    ps_v = psum.tile([128, 512], F32, tag="ps")
                    for ht in range(HT):
                        nc.tensor.matmul(
                            ps_v[:, : HPC * 128],
                            hid_slice(ht, stl * 128, (stl + 1) * 128),
                            wv_sb[:, ht, :],
                            start=(ht == 0), stop=(ht == HT - 1),
                        )
                    nc.vector.tensor_tensor(
                        v_sb[:, st, :], ps_v[:, : HPC * 128], b_v_sb[:],
                        op=mybir.AluOpType.add,
                    )

                # cos/sin slices for this chunk (scaled tables for q)
                csq = cs_pool.tile([128, 2, 512], F32, tag="csq")
                nc.scalar.dma_start(csq[:], cs_q[:, :, lo:lo + 512])
                csk = cs_pool.tile([128, 2, 512], F32, tag="csk")
                nc.scalar.dma_start(csk[:], cs_k[:, :, lo:lo + 512])

                # q/k part: transposed orientation [d, s]
                for ct in range(QK_TILES):
                    n, is_k = ct // 2, ct % 2
                    wt = wqk_pool.tile([128, HT, 128], BF16, tag="wt")
                    (nc.sync if ct % 2 else nc.scalar).dma_start(wt[:], w_qk[ct])
                    cst = csk if is_k else csq
                    dest = kT_sb if is_k else qT_sb
                    ps_qk = psum.tile([128, 512], F32, tag="ps")
                    for ht in range(HT):
                        nc.tensor.matmul(
                            ps_qk[:],
                            wt[:, ht, :],
                            hid_slice(ht, 0, 512),
                            start=(ht == 0), stop=(ht == HT - 1),
                        )
                    qf = rt_pool.tile([128, 512], F32, tag="qf")
                    nc.scalar.activation(
                        qf[:], ps_qk[:], mybir.ActivationFunctionType.Identity,
                        bias=b_qk_sb[:, ct:ct + 1],
                    )
                    qs = rt_pool.tile([128, 512], F32, tag="qs")
                    nc.gpsimd.dma_start(qs[0:32, :], qf[32:64, :])
                    nc.gpsimd.dma_start(qs[32:64, :], qf[0:32, :])
                    nc.gpsimd.dma_start(qs[64:96, :], qf[96:128, :])
                    nc.gpsimd.dma_start(qs[96:128, :], qf[64:96, :])
                    t1 = rt_pool.tile([128, 512], F32, tag="t1")
                    nc.gpsimd.tensor_tensor(
                        t1[:], qs[:], cst[:, 1, :], op=mybir.AluOpType.mult)
                    t2 = rt_pool.tile([128, 512], F32, tag="t2")
                    nc.vector.tensor_tensor(
                        t2[:], qf[:], cst[:, 0, :], op=mybir.AluOpType.mult)
                    nc.vector.tensor_tensor(
                        dest[:, n, lo:lo + 512], t1[:], t2[:],
                        op=mybir.AluOpType.add)

        if debug:
            nc.sync.dma_start(dbg_q[:], qT_sb[:])
            nc.sync.dma_start(dbg_k[:], kT_sb[:])
            nc.sync.dma_start(dbg_v[:], v_sb[:])

        # ---------------- Phase 2+3: attention + dense ----------------
        if phases < 2:
            nc.compile()
            return nc
        with ExitStack() as actx:
            attnw = actx.enter_context(tc.tile_pool(name="attnw", bufs=1))
            wdT_sb = attnw.tile([128, HPC, H], BF16)
            for n in range(HPC):
                nc.scalar.dma_start(wdT_sb[:, n, :], wdT[n])
            ctx_sb = attnw.tile([128, HPC, S], BF16)

            atmp_ctx = ExitStack()
            attn = atmp_ctx.enter_context(tc.tile_pool(name="attn", bufs=1))
            mt_pool = atmp_ctx.enter_context(tc.tile_pool(name="mt", bufs=2))
            ctx_psum = atmp_ctx.enter_context(tc.tile_pool(name="ctxps", bufs=1, space="PSUM"))

            NSEG = 4
            W = S // NSEG
            CPS = W // 512 if W >= 512 else 1  # 512-chunks per segment
            dout = atmp_ctx.enter_context(tc.tile_pool(name="dout", bufs=3))
            dps = atmp_ctx.enter_context(tc.tile_pool(name="dps", bufs=2, space="PSUM"))
            for h2 in range(NSEG):
                for n in range(HPC):
                    sLo, sHi = W * h2, W * (h2 + 1)
                    PTh = attn.tile([128, ST, W], BF16, tag="PT", bufs=3, name="PTh")
                    for t in range(ST):
                        t0 = s0_of(t)
                        if t0 >= sHi:
                            continue
                        s = max(t0, sLo)
                        while s < sHi:
                            w = min(512, sHi - s)
                            ps_s = psum.tile([128, 512], F32, tag="ps", name="ps_s")
                            nc.tensor.matmul(
                                ps_s[:, :w],
                                kT_sb[:, n, 128 * t:128 * (t + 1)],
                                qT_sb[:, n, s:s + w],
                                start=True, stop=True,
                            )
                            nc.scalar.activation(
                                PTh[:, t, s - sLo:s - sLo + w], ps_s[:, :w],
                                mybir.ActivationFunctionType.Exp, scale=coeff)
                            s += w
                        if mode == "causal" and sLo <= t0:
                            if t0 % 512:
                                c0 = t0 - (t0 % 512)
                                nc.vector.memset(PTh[:, t, c0 - sLo:t0 - sLo], 0.0)
                            nc.vector.tensor_tensor(
                                PTh[:, t, t0 - sLo:t0 - sLo + 128],
                                PTh[:, t, t0 - sLo:t0 - sLo + 128], triu_sb[:],
                                op=mybir.AluOpType.mult)
                        elif mode == "general":
                            mt = mt_pool.tile([128, W], BF16, tag="m01")
                            nc.sync.dma_start(mt[:], m01[t][:, sLo:sHi])
                            nc.vector.tensor_tensor(
                                PTh[:, t, :], PTh[:, t, :], mt[:],
                                op=mybir.AluOpType.mult)

                    # row sums for this segment, then 1/l broadcast
                    l_sb = attn.tile([1, W], F32, tag="l", bufs=2, name="l_sb")
                    for cc in range(CPS):
                        c = CPS * h2 + cc
                        tmax = min(ST - 1, 4 * c + 3) if mode == "causal" else ST - 1
                        ps_l = psum.tile([1, 512], F32, tag="ps", name="ps_l")
                        for t in range(tmax + 1):
                            nc.tensor.matmul(
                                ps_l[:], ones_sb[:], PTh[:, t, 512 * cc:512 * (cc + 1)],
                                start=(t == 0), stop=(t == tmax))
                        nc.scalar.copy(l_sb[:, 512 * cc:512 * (cc + 1)], ps_l[:])
                    linv = attn.tile([1, W], F32, tag="linv", bufs=2, name="linv")
                    nc.vector.reciprocal(linv[:], l_sb[:])
                    l_dram = scratch.tile([1, W], F32, tag="ldram")
                    nc.sync.dma_start(l_dram[:], linv[:])
                    linv_b = attn.tile([128, W], F32, tag="linvb", bufs=2, name="linv_b")
                    nc.scalar.dma_start(linv_b[:], l_dram[:].to_broadcast((128, W)))

                    # ctxT[d, s] accumulation over t tiles (this half)
                    ctx_ps = ctx_psum.tile([128, W], F32, tag="ctx", bufs=2, name="ctx_ps")
                    for cc in range(CPS):
                        c = CPS * h2 + cc
                        tmax = min(ST - 1, 4 * c + 3) if mode == "causal" else ST - 1
                        for t in range(tmax + 1):
                            nc.tensor.matmul(
                                ctx_ps[:, 512 * cc:512 * (cc + 1)],
                                v_sb[:, t, 128 * n:128 * (n + 1)],
                                PTh[:, t, 512 * cc:512 * (cc + 1)],
                                start=(t == 0), stop=(t == tmax))
                    nc.vector.tensor_tensor(
                        ctx_sb[:, n, sLo:sHi], ctx_ps[:], linv_b[:],
                        op=mybir.AluOpType.mult)

                # dense for this segment's s tiles (ctx complete across all heads)
                if phases >= 3:
                    for stl in range(W // 128):
                        st = (W * h2) // 128 + stl
                        ot = dout.tile([128, 8, 512], F16, tag="ot")
                        for ch in range(8):
                            ps_o = dps.tile([128, 512], F32, tag="dp", name="ps_o")
                            for nn in range(HPC):
                                nc.tensor.matmul(
                                    ps_o[:],
                                    ctx_sb[:, nn, 128 * st:128 * (st + 1)],
                                    wdT_sb[:, nn, 512 * ch:512 * (ch + 1)],
                                    start=(nn == 0), stop=(nn == HPC - 1))
                            if ch % 2 == 0:
                                nc.scalar.copy(ot[:, ch, :], ps_o[:])
                            else:
                                nc.vector.tensor_copy(ot[:, ch, :], ps_o[:])
                        eng = nc.scalar if st % 2 == 0 else nc.sync
                        eng.dma_start(
                            out_p[128 * st:128 * (st + 1), :],
                            ot[:].rearrange("p c s -> p (c s)"))

            if debug:
                nc.sync.dma_start(dbg_c[:], ctx_sb[:])

            atmp_ctx.close()

    nc.compile()
    return nc


def _prep_inputs(hidden_states, position_ids, attention_mask, layer_id, Wqkv, bqkv, Wd):
    coeff = float(np.asarray(layer_id).item() + 1)
    m = np.asarray(attention_mask).reshape(S, S)
    if not m.any():
        mode = "none"
    elif bool((m == np.triu(np.ones((S, S), bool), 1)).all()):
        mode = "causal"
    else:
        mode = "general"

    hs = np.asarray(hidden_states, np.float32).reshape(S, H)
    hT = np.ascontiguousarray(hs.T.reshape(HT, 128, S).transpose(1, 0, 2)).astype(BF)

    # RoPE tables (match reference fp32 math)
    inv_freq = (1.0 / (10000.0 ** (np.arange(0, ROT, 2, dtype=np.float32) / ROT))).astype(np.float32)
    freqs = np.arange(S, dtype=np.float32)[:, None] * inv_freq[None, :]
    emb = np.concatenate([freqs, freqs], axis=-1)          # [S, 64]
    cos_t, sin_t = np.cos(emb), np.sin(emb)                # fp32 [S, 64]
    pid = np.asarray(position_ids)[0, 0, :].astype(np.int64)
    bid = np.asarray(position_ids)[0, 1, :].astype(np.int64)
    cp, sp = cos_t[pid].T, sin_t[pid].T                    # [64, S]
    cb, sb_ = cos_t[bid].T, sin_t[bid].T
    cos_full = np.concatenate([cp, cb], 0)                 # [128, S]
    # rope via partition-swapped copy: t1[p] = q[p^32] * sin_eff[p]
    sin_eff = np.concatenate([-sp[:32], sp[32:], -sb_[:32], sb_[32:]], 0)
    qscale = np.float32(1.0 / (math.sqrt(HD) * coeff))
    cs_q = np.stack([cos_full * qscale, sin_eff * qscale], 1).astype(np.float32)  # [128,2,S]
    cs_k = np.stack([cos_full, sin_eff], 1).astype(np.float32)

    tri = np.triu(np.ones((128, 128), np.float32)).astype(BF)  # t<=s allowed

    Wq = np.asarray(Wqkv, np.float32).reshape(NH, 3, HD, H)
    bq = np.asarray(bqkv, np.float32).reshape(NH, 3, HD)
    Wd_ = np.asarray(Wd, np.float32)

    per_core = []
    for c in range(NCORES):
        heads = slice(4 * c, 4 * c + 4)
        wqk = Wq[heads, 0:2]                               # [4, 2, 128, H]
        # -> [ct=8, p=128, ht=32, j=128]
        wqk = wqk.reshape(8, HD, HT, 128).transpose(0, 3, 2, 1)   # [8, p, ht, j]
        w_qk = np.ascontiguousarray(wqk).astype(BF)
        wv = Wq[heads, 2]                                  # [4, 128, H]
        wv = wv.reshape(4 * HD, HT, 128).transpose(1, 2, 0)  # [ht, p, 512]
        wv = wv.transpose(1, 0, 2)                         # [p, ht, 512]
        w_v = np.ascontiguousarray(wv).astype(BF)
        b_qk = np.ascontiguousarray(bq[heads, 0:2].reshape(8, 128)).astype(np.float32)
        b_v = np.broadcast_to(bq[heads, 2].reshape(1, 512), (128, 512)).astype(np.float32)
        wd = Wd_[:, 512 * c:512 * (c + 1)]                 # [H, 512]
        wd = np.ascontiguousarray(wd.T.reshape(HPC, 128, H)).astype(BF)
        im = {
            "hT": hT, "w_qk": w_qk, "w_v": w_v, "b_qk": b_qk, "b_v": b_v,
            "cs_q": cs_q, "cs_k": cs_k, "triu": tri, "wdT": wd,
        }
        if mode == "general":
            m01 = (~m).astype(np.float32).T  # [t, s] 1=allowed
            im["m01"] = np.ascontiguousarray(m01.reshape(ST, 128, S)).astype(BF)
        per_core.append(im)
    return coeff, mode, per_core


def kernel(hidden_states, position_ids, attention_mask, layer_id, Wqkv, bqkv, Wd, bd):
    coeff, mode, per_core = _prep_inputs(
        hidden_states, position_ids, attention_mask, layer_id, Wqkv, bqkv, Wd)
    key = (coeff, mode)
    if key not in _PROG_CACHE:
        _PROG_CACHE[key] = _build_program(coeff, mode)
    nc = _PROG_CACHE[key]
    res = run_bass_kernel_spmd(nc, per_core, core_ids=list(range(NCORES)))
    out = np.zeros((S, H), np.float64)
    for r in res.results:
        out += r["out_p"].astype(np.float64)
    out += np.asarray(bd, np.float32).astype(np.float64)
    return out.astype(np.float32).reshape(S, B, H)
